# revision 6
# baseline (speedup 1.0000x reference)
"""Trainium2 Bass kernel for nn_Attention (B=2, S=2048, D=1024, H=16).

Sharding: 8 cores = 2 batches x 4 head-groups (4 heads per core).
Each core computes QKV projection for its batch restricted to its 4 heads,
full (non-causal) attention for those heads, and a partial output
projection over its 256 channels. The host sums the 4 partial outputs per
batch (the out-proj bias is fed only to head-group 0's cores).

Device dataflow (per core), matmuls in float32r (~1.5e-4 rel err, 4x the
fp32 PE throughput):
  1. x [2048,1024] -> PE-transpose -> xT [f, tok] (f32r)
  2. qk proj (natural [tok, 512]): 8 accum matmuls + K=1 ones-row bias
     -> RoPE on DVE (writes evens/odds-permuted d order) -> PE-transpose
     -> qT/kT [128, 2 head-pair chunks, 2048]
     v proj -> V [kt, 16 tiles, 4 heads, 65] with a ones column (col 64)
  3. attention per (qt-chunk, head-pair): logitsT = kT.T @ qT (row-packed
     2 heads at K=64), exp on ACT (1/8 scale fused), OT accum =
     V_aug.T @ ET over 16 kt tiles; row 64 of OT = softmax sums.
  4. divide: recip(sums) -> PE outer-product broadcast -> DVE multiply;
     odd head rows shifted to partitions 64:128 via SBUF-SBUF DMA.
  5. out-proj: y[tok,e] accum over 2 channel chunks + K=1 ones-row bias.
"""

import numpy as np

S = 2048
D = 1024
HD = 64
H_LOC = 4  # heads per core
N_CORES = 8
TT = 16  # token tiles of 128
G = 4    # token groups of 512
QC = 4   # query chunks of 512
KT = 16  # key tiles of 128

_CACHED = {}


def build_nc(repeats: int = 1):
    import concourse.bass as bass
    import concourse.mybir as mybir
    from concourse import bacc
    from concourse.tile import TileContext
    from concourse.masks import make_identity

    f32 = mybir.dt.float32
    f32r = mybir.dt.float32r
    Exp = mybir.ActivationFunctionType.Exp

    nc = bacc.Bacc("TRN2", target_bir_lowering=False, debug=False,
                   num_devices=N_CORES)

    x_d = nc.dram_tensor("x", [S, D], f32, kind="ExternalInput")
    cos_d = nc.dram_tensor("cosr", [S, 128], f32, kind="ExternalInput")
    sin_d = nc.dram_tensor("sinr", [S, 128], f32, kind="ExternalInput")
    wqk_d = nc.dram_tensor("wqk", [D, 512], f32r, kind="ExternalInput")
    wv_d = nc.dram_tensor("wv", [D, 256], f32r, kind="ExternalInput")
    wout_d = nc.dram_tensor("wout", [256, D], f32r, kind="ExternalInput")
    bqk_d = nc.dram_tensor("bqk", [1, 512], f32r, kind="ExternalInput")
    bv_d = nc.dram_tensor("bv", [1, 256], f32r, kind="ExternalInput")
    bout_d = nc.dram_tensor("bout", [1, D], f32r, kind="ExternalInput")
    ones_d = nc.dram_tensor("ones", [1, 128], f32r, kind="ExternalInput")
    onescol_d = nc.dram_tensor("onescol", [128, TT, H_LOC, 1], f32r,
                               kind="ExternalInput")
    y_d = nc.dram_tensor("y", [S, D], f32, kind="ExternalOutput")

    with TileContext(nc) as tc:
        with (
            tc.tile_pool(name="const", bufs=1) as cpool,
            tc.tile_pool(name="xin", bufs=1) as xpool,
            tc.tile_pool(name="xt", bufs=1) as xtpool,
            tc.tile_pool(name="qkr", bufs=1) as qkrpool,
            tc.tile_pool(name="rtmp", bufs=2) as rtpool,
            tc.tile_pool(name="big", bufs=1) as bigpool,
            tc.tile_pool(name="et", bufs=4) as etpool,
            tc.tile_pool(name="yt", bufs=2) as ypool,
            tc.tile_pool(name="sml", bufs=2) as spool,
            tc.tile_pool(name="psp", bufs=4, space="PSUM") as psp,
            tc.tile_pool(name="psl", bufs=2, space="PSUM") as psl,
            tc.tile_pool(name="pso", bufs=2, space="PSUM") as pso,
        ):
            # ---- constants / weights ----
            wqk_sb = cpool.tile([128, 8, 512], f32r)
            wv_sb = cpool.tile([128, 8, 256], f32r)
            wout_sb = cpool.tile([128, 2, D], f32r)
            cos_sb = cpool.tile([128, TT, 128], f32)
            sin_sb = cpool.tile([128, TT, 128], f32)
            bqk_sb = cpool.tile([1, 512], f32r)
            bv_sb = cpool.tile([1, 256], f32r)
            bout_sb = cpool.tile([1, D], f32r)
            ones_sb = cpool.tile([1, 128], f32r)
            ident = cpool.tile([128, 128], f32)

            nc.sync.dma_start(wqk_sb[:], wqk_d.ap().rearrange("(i p) c -> p i c", p=128))
            nc.sync.dma_start(wv_sb[:], wv_d.ap().rearrange("(i p) c -> p i c", p=128))
            nc.sync.dma_start(wout_sb[:], wout_d.ap().rearrange("(i p) c -> p i c", p=128))
            nc.sync.dma_start(cos_sb[:], cos_d.ap().rearrange("(t p) c -> p t c", p=128))
            nc.sync.dma_start(sin_sb[:], sin_d.ap().rearrange("(t p) c -> p t c", p=128))
            nc.sync.dma_start(bqk_sb[:], bqk_d[:])
            nc.sync.dma_start(bv_sb[:], bv_d[:])
            nc.sync.dma_start(bout_sb[:], bout_d[:])
            nc.sync.dma_start(ones_sb[:], ones_d[:])
            make_identity(nc, ident[:])

            def body(_iv=None):
                qT = bigpool.tile([128, 2, S], f32r, tag="qT")
                kT = bigpool.tile([128, 2, S], f32r, tag="kT")
                attn = bigpool.tile([128, 2, S], f32r, tag="attn")
                v_sb = bigpool.tile([128, TT, H_LOC, 65], f32r, tag="v")
                nc.sync.dma_start(v_sb[:, :, :, 64:65], onescol_d[:])

                # ================= stage A: projections =================
                for g in range(G):
                    xts = []
                    for ti in range(4):
                        tt = g * 4 + ti
                        x_t = xpool.tile([128, D], f32, tag=f"x{ti}")
                        nc.sync.dma_start(x_t[:], x_d[tt * 128:(tt + 1) * 128, :])
                        xts.append(x_t)

                    xT_g = xtpool.tile([128, 8, 512], f32r)
                    for fc in range(8):
                        ps_x = psp.tile([128, 512], f32, tag="pp")
                        for ti in range(4):
                            nc.tensor.transpose(
                                ps_x[:, ti * 128:(ti + 1) * 128],
                                xts[ti][:, fc * 128:(fc + 1) * 128], ident[:])
                        nc.vector.tensor_copy(xT_g[:, fc, :], ps_x[:])

                    qkrs = []
                    for ti in range(4):
                        tt = g * 4 + ti
                        # ---- v projection ----
                        ps_v = psp.tile([128, 256], f32, tag="pp")
                        for fc in range(8):
                            nc.tensor.matmul(
                                ps_v[:], xT_g[:, fc, ti * 128:(ti + 1) * 128],
                                wv_sb[:, fc, :],
                                start=(fc == 0), stop=False)
                        nc.tensor.matmul(ps_v[:], ones_sb[0:1, 0:128], bv_sb[:],
                                         start=False, stop=True)
                        nc.vector.tensor_copy(
                            v_sb[:, tt, :, 0:64],
                            ps_v[:].rearrange("p (h d) -> p h d", h=H_LOC))

                        # ---- qk projection (natural layout) ----
                        ps_qk = psp.tile([128, 512], f32, tag="pp")
                        for fc in range(8):
                            nc.tensor.matmul(
                                ps_qk[:], xT_g[:, fc, ti * 128:(ti + 1) * 128],
                                wqk_sb[:, fc, :],
                                start=(fc == 0), stop=False)
                        nc.tensor.matmul(ps_qk[:], ones_sb[0:1, 0:128], bqk_sb[:],
                                         start=False, stop=True)

                        # ---- rope (DVE), writes evens/odds-permuted ----
                        qk_r = qkrpool.tile([128, 512], f32, tag=f"qkr{ti}")
                        cos4 = cos_sb[:, tt, :].rearrange("p (h j) -> p h j", h=4)
                        sin4 = sin_sb[:, tt, :].rearrange("p (h j) -> p h j", h=4)
                        for half in range(2):  # 0: q cols 0:256, 1: k cols
                            src = ps_qk[:, half * 256:(half + 1) * 256].rearrange(
                                "p (h j two) -> p two h j", j=32, two=2)
                            dst = qk_r[:, half * 256:(half + 1) * 256].rearrange(
                                "p (h pm j) -> p pm h j", pm=2, j=32)
                            ev, od = src[:, 0], src[:, 1]
                            t1 = rtpool.tile([128, 4, 32], f32, tag="t1")
                            t2 = rtpool.tile([128, 4, 32], f32, tag="t2")
                            nc.vector.tensor_mul(t1[:], od, sin4)
                            nc.vector.tensor_mul(dst[:, 0], ev, cos4)
                            nc.vector.tensor_sub(dst[:, 0], dst[:, 0], t1[:])
                            nc.vector.tensor_mul(t2[:], ev, sin4)
                            nc.vector.tensor_mul(dst[:, 1], od, cos4)
                            nc.vector.tensor_add(dst[:, 1], dst[:, 1], t2[:])
                        qkrs.append(qk_r)

                    # ---- transpose roped qk into qT/kT ----
                    for cc in range(4):
                        ps_t = psp.tile([128, 512], f32, tag="pp")
                        for ti in range(4):
                            nc.tensor.transpose(
                                ps_t[:, ti * 128:(ti + 1) * 128],
                                qkrs[ti][:, cc * 128:(cc + 1) * 128], ident[:])
                        dstbuf = qT if cc < 2 else kT
                        nc.vector.tensor_copy(
                            dstbuf[:, cc % 2, g * 512:(g + 1) * 512], ps_t[:])

                # ================= stage B: attention =================
                for qc in range(QC):
                    for hp in range(2):
                        O_A = pso.tile([128, 512], f32, tag="O")
                        O_B = pso.tile([128, 512], f32, tag="O")
                        for kt in range(KT):
                            ps_lA = psl.tile([128, 512], f32, tag="L")
                            ps_lB = psl.tile([128, 512], f32, tag="L")
                            nc.tensor.matmul(
                                ps_lA[:],
                                kT[0:64, hp, kt * 128:(kt + 1) * 128],
                                qT[0:64, hp, qc * 512:(qc + 1) * 512],
                                start=True, stop=True)
                            nc.tensor.matmul(
                                ps_lB[:],
                                kT[64:128, hp, kt * 128:(kt + 1) * 128],
                                qT[64:128, hp, qc * 512:(qc + 1) * 512],
                                start=True, stop=True)
                            etA = etpool.tile([128, 512], f32r, tag="et")
                            etB = etpool.tile([128, 512], f32r, tag="et")
                            nc.scalar.activation(etA[:], ps_lA[:], Exp, scale=0.125)
                            nc.scalar.activation(etB[:], ps_lB[:], Exp, scale=0.125)
                            nc.tensor.matmul(
                                O_A[0:65, :], v_sb[:, kt, 2 * hp, :], etA[:],
                                start=(kt == 0), stop=(kt == KT - 1))
                            nc.tensor.matmul(
                                O_B[0:65, :], v_sb[:, kt, 2 * hp + 1, :], etB[:],
                                start=(kt == 0), stop=(kt == KT - 1))
                        for (O_ps, odd) in ((O_A, 0), (O_B, 1)):
                            recip = spool.tile([1, 512], f32r, tag="rc")
                            with nc.allow_low_precision(
                                    reason="f32r reciprocal feeds f32r matmul"):
                                nc.vector.reciprocal(recip[:], O_ps[64:65, :])
                            bc_ps = psl.tile([128, 512], f32, tag="L")
                            nc.tensor.matmul(bc_ps[0:64, :], ones_sb[0:1, 0:64],
                                             recip[:], start=True, stop=True)
                            bc_sb = spool.tile([64, 512], f32, tag="bc")
                            nc.vector.tensor_copy(bc_sb[:], bc_ps[0:64, :])
                            if not odd:
                                nc.vector.tensor_mul(
                                    attn[0:64, hp, qc * 512:(qc + 1) * 512],
                                    O_ps[0:64, :], bc_sb[:])
                            else:
                                t_at = spool.tile([64, 512], f32r, tag="ta")
                                nc.vector.tensor_mul(t_at[:], O_ps[0:64, :], bc_sb[:])
                                nc.sync.dma_start(
                                    attn[64:128, hp, qc * 512:(qc + 1) * 512],
                                    t_at[:])

                    # ---- stage C: out-proj for this query chunk ----
                    for ti in range(4):
                        tt = qc * 4 + ti
                        y_t = ypool.tile([128, D], f32)
                        for ec in range(2):
                            ps_y = psp.tile([128, 512], f32, tag="pp")
                            nc.tensor.matmul(
                                ps_y[:], attn[:, 0, tt * 128:(tt + 1) * 128],
                                wout_sb[:, 0, ec * 512:(ec + 1) * 512],
                                start=True, stop=False)
                            nc.tensor.matmul(
                                ps_y[:], attn[:, 1, tt * 128:(tt + 1) * 128],
                                wout_sb[:, 1, ec * 512:(ec + 1) * 512],
                                start=False, stop=False)
                            nc.tensor.matmul(
                                ps_y[:], ones_sb[0:1, 0:128],
                                bout_sb[0:1, ec * 512:(ec + 1) * 512],
                                start=False, stop=True)
                            nc.vector.tensor_copy(
                                y_t[:, ec * 512:(ec + 1) * 512], ps_y[:])
                        nc.sync.dma_start(y_d[tt * 128:(tt + 1) * 128, :], y_t[:])

            if repeats == 1:
                body()
            else:
                with tc.For_i(0, repeats, 1) as _i:
                    body(_i)

    nc.compile()
    return nc


def _prep_in_maps(x, rope_cos, rope_sin, W_qkv, b_qkv, W_out, b_out):
    f32 = np.float32
    W3 = np.asarray(W_qkv, dtype=f32).reshape(D, 16, 3, HD)  # [f, head, qkv, d]
    b3 = np.asarray(b_qkv, dtype=f32).reshape(16, 3, HD)
    cos_r = np.ascontiguousarray(np.tile(np.asarray(rope_cos, dtype=f32), (1, 4)))
    sin_r = np.ascontiguousarray(np.tile(np.asarray(rope_sin, dtype=f32), (1, 4)))
    ones = np.ones((1, 128), dtype=f32)
    onescol = np.ones((128, TT, H_LOC, 1), dtype=f32)
    W_out = np.asarray(W_out, dtype=f32)
    b_out = np.asarray(b_out, dtype=f32)
    x = np.asarray(x, dtype=f32)

    in_maps = []
    for c in range(N_CORES):
        b, hg = divmod(c, 4)
        hs = slice(hg * H_LOC, (hg + 1) * H_LOC)
        wq = W3[:, hs, 0, :].reshape(D, 256)
        wk = W3[:, hs, 1, :].reshape(D, 256)
        wv = W3[:, hs, 2, :].reshape(D, 256)
        bq = b3[hs, 0, :].reshape(1, 256)
        bk = b3[hs, 1, :].reshape(1, 256)
        bv = b3[hs, 2, :].reshape(1, 256)
        in_maps.append({
            "x": np.ascontiguousarray(x[b]),
            "cosr": cos_r, "sinr": sin_r,
            "wqk": np.ascontiguousarray(np.concatenate([wq, wk], axis=1)),
            "wv": np.ascontiguousarray(wv),
            "wout": np.ascontiguousarray(W_out[hg * 256:(hg + 1) * 256, :]),
            "bqk": np.ascontiguousarray(np.concatenate([bq, bk], axis=1)),
            "bv": np.ascontiguousarray(bv),
            "bout": (np.ascontiguousarray(b_out.reshape(1, D)) if hg == 0
                     else np.zeros((1, D), dtype=f32)),
            "ones": ones, "onescol": onescol,
        })
    return in_maps


def kernel(x, rope_cos, rope_sin, W_qkv, b_qkv, W_out, b_out):
    from concourse.bass_utils import run_bass_kernel_spmd

    if "nc" not in _CACHED:
        _CACHED["nc"] = build_nc(1)
    nc = _CACHED["nc"]
    in_maps = _prep_in_maps(x, rope_cos, rope_sin, W_qkv, b_qkv, W_out, b_out)
    res = run_bass_kernel_spmd(nc, in_maps, list(range(N_CORES)))
    B = x.shape[0]
    out = np.zeros((B, S, D), dtype=np.float32)
    for c in range(N_CORES):
        b = c // 4
        out[b] += res.results[c]["y"]
    return out


# revision 9
# speedup vs baseline: 15.4077x; 15.4077x over previous
"""Trainium2 Bass kernel for nn_Attention (B=2, S=2048, D=1024, H=16).

Sharding: 8 cores = 2 batches x 4 head-groups (4 heads per core).
Each core computes QKV projection for its batch restricted to its 4 heads,
full (non-causal) attention for those heads, and a partial output
projection over its 256 channels. The host sums the 4 partial outputs per
batch (the out-proj bias is fed only to head-group 0's cores).

Device dataflow (per core), matmuls in float32r (~1.5e-4 rel err, 4x the
fp32 PE throughput):
  1. x [2048,1024] -> PE-transpose -> xT [f, tok] (f32r)
  2. qk proj (natural [tok, 512]): 8 accum matmuls + K=1 ones-row bias
     -> RoPE on DVE (writes evens/odds-permuted d order) -> PE-transpose
     -> qT/kT [128, 2 head-pair chunks, 2048]
     v proj -> V [kt, 16 tiles, 4 heads, 65] with a ones column (col 64)
  3. attention per (qt-chunk, head-pair): logitsT = kT.T @ qT (row-packed
     2 heads at K=64), exp on ACT (1/8 scale fused), OT accum =
     V_aug.T @ ET over 16 kt tiles; row 64 of OT = softmax sums.
  4. divide: recip(sums) -> PE outer-product broadcast -> DVE multiply;
     odd head rows shifted to partitions 64:128 via SBUF-SBUF DMA.
  5. out-proj: y[tok,e] accum over 2 channel chunks + K=1 ones-row bias.
"""

import numpy as np

S = 2048
D = 1024
HD = 64
H_LOC = 4  # heads per core
N_CORES = 8
TT = 16  # token tiles of 128
G = 4    # token groups of 512
QC = 4   # query chunks of 512
KT = 16  # key tiles of 128

_CACHED = {}


def build_nc(repeats: int = 1):
    import concourse.bass as bass
    import concourse.mybir as mybir
    from concourse import bacc
    from concourse.tile import TileContext
    from concourse.masks import make_identity

    f32 = mybir.dt.float32
    f32r = mybir.dt.float32r
    Exp = mybir.ActivationFunctionType.Exp

    nc = bacc.Bacc("TRN2", target_bir_lowering=False, debug=False,
                   num_devices=N_CORES)

    x_d = nc.dram_tensor("x", [S, D], f32, kind="ExternalInput")
    cos_d = nc.dram_tensor("cosr", [S, 128], f32, kind="ExternalInput")
    sin_d = nc.dram_tensor("sinr", [S, 128], f32, kind="ExternalInput")
    wqk_d = nc.dram_tensor("wqk", [D, 512], f32r, kind="ExternalInput")
    wv_d = nc.dram_tensor("wv", [D, 256], f32r, kind="ExternalInput")
    wout_d = nc.dram_tensor("wout", [256, D], f32r, kind="ExternalInput")
    bqk_d = nc.dram_tensor("bqk", [1, 512], f32r, kind="ExternalInput")
    bv_d = nc.dram_tensor("bv", [1, 256], f32r, kind="ExternalInput")
    bout_d = nc.dram_tensor("bout", [1, D], f32r, kind="ExternalInput")
    ones_d = nc.dram_tensor("ones", [1, 128], f32r, kind="ExternalInput")
    onescol_d = nc.dram_tensor("onescol", [128, 64], f32r,
                               kind="ExternalInput")
    y_d = nc.dram_tensor("y", [S, D], f32, kind="ExternalOutput")

    with TileContext(nc) as tc:
        with (
            tc.tile_pool(name="const", bufs=1) as cpool,
            tc.tile_pool(name="xin", bufs=1) as xpool,
            tc.tile_pool(name="xt", bufs=1) as xtpool,
            tc.tile_pool(name="qkr", bufs=1) as qkrpool,
            tc.tile_pool(name="rtmp", bufs=2) as rtpool,
            tc.tile_pool(name="big", bufs=1) as bigpool,
            tc.tile_pool(name="et", bufs=2) as etpool,
            tc.tile_pool(name="yt", bufs=2) as ypool,
            tc.tile_pool(name="sml", bufs=2) as spool,
            tc.tile_pool(name="psp", bufs=3, space="PSUM") as psp,
            tc.tile_pool(name="psl", bufs=1, space="PSUM") as psl,
            tc.tile_pool(name="pso", bufs=2, space="PSUM") as pso,
        ):
            # ---- constants / weights ----
            wqk_sb = cpool.tile([128, 8, 512], f32r)
            wv_sb = cpool.tile([128, 8, 256], f32r)
            wout_sb = cpool.tile([128, 2, D], f32r)
            cos_sb = cpool.tile([128, TT, 128], f32)
            sin_sb = cpool.tile([128, TT, 128], f32)
            bqk_sb = cpool.tile([1, 512], f32r)
            bv_sb = cpool.tile([1, 256], f32r)
            bout_sb = cpool.tile([1, D], f32r)
            ones_sb = cpool.tile([1, 128], f32r)
            onescol_sb = cpool.tile([128, 64], f32r)
            ident = cpool.tile([128, 128], f32)

            nc.sync.dma_start(wqk_sb[:], wqk_d.ap().rearrange("(i p) c -> p i c", p=128))
            nc.sync.dma_start(wv_sb[:], wv_d.ap().rearrange("(i p) c -> p i c", p=128))
            nc.sync.dma_start(wout_sb[:], wout_d.ap().rearrange("(i p) c -> p i c", p=128))
            nc.sync.dma_start(cos_sb[:], cos_d.ap().rearrange("(t p) c -> p t c", p=128))
            nc.sync.dma_start(sin_sb[:], sin_d.ap().rearrange("(t p) c -> p t c", p=128))
            nc.sync.dma_start(bqk_sb[:], bqk_d[:])
            nc.sync.dma_start(bv_sb[:], bv_d[:])
            nc.sync.dma_start(bout_sb[:], bout_d[:])
            nc.sync.dma_start(ones_sb[:], ones_d[:])
            nc.sync.dma_start(onescol_sb[:], onescol_d[:])
            make_identity(nc, ident[:])

            def body(_iv=None):
                qT = bigpool.tile([128, 2, S], f32r, tag="qT")
                kT = bigpool.tile([128, 2, S], f32r, tag="kT")
                attn = bigpool.tile([128, 2, S], f32r, tag="attn")
                v_sb = bigpool.tile([128, TT, H_LOC, 65], f32r, tag="v")
                nc.vector.tensor_copy(
                    v_sb[:, :, :, 64:65],
                    onescol_sb[:].rearrange("p (t h o) -> p t h o", h=H_LOC, o=1))

                # ================= stage A: projections =================
                for g in range(G):
                    xts = []
                    for ti in range(4):
                        tt = g * 4 + ti
                        x_t = xpool.tile([128, D], f32, tag=f"x{ti}")
                        nc.sync.dma_start(x_t[:], x_d[tt * 128:(tt + 1) * 128, :])
                        xts.append(x_t)

                    xT_g = xtpool.tile([128, 8, 512], f32r)
                    for fc in range(8):
                        ps_x = psp.tile([128, 512], f32, tag="pp")
                        for ti in range(4):
                            nc.tensor.transpose(
                                ps_x[:, ti * 128:(ti + 1) * 128],
                                xts[ti][:, fc * 128:(fc + 1) * 128], ident[:])
                        nc.vector.tensor_copy(xT_g[:, fc, :], ps_x[:])

                    qkrs = []
                    for ti in range(4):
                        tt = g * 4 + ti
                        # ---- v projection ----
                        ps_v = psp.tile([128, 256], f32, tag="pp")
                        for fc in range(8):
                            nc.tensor.matmul(
                                ps_v[:], xT_g[:, fc, ti * 128:(ti + 1) * 128],
                                wv_sb[:, fc, :],
                                start=(fc == 0), stop=False)
                        nc.tensor.matmul(ps_v[:], ones_sb[0:1, 0:128], bv_sb[:],
                                         start=False, stop=True)
                        nc.vector.tensor_copy(
                            v_sb[:, tt, :, 0:64],
                            ps_v[:].rearrange("p (h d) -> p h d", h=H_LOC))

                        # ---- qk projection (natural layout) ----
                        ps_qk = psp.tile([128, 512], f32, tag="pp")
                        for fc in range(8):
                            nc.tensor.matmul(
                                ps_qk[:], xT_g[:, fc, ti * 128:(ti + 1) * 128],
                                wqk_sb[:, fc, :],
                                start=(fc == 0), stop=False)
                        nc.tensor.matmul(ps_qk[:], ones_sb[0:1, 0:128], bqk_sb[:],
                                         start=False, stop=True)

                        # ---- rope (DVE), writes evens/odds-permuted ----
                        qk_r = qkrpool.tile([128, 512], f32, tag=f"qkr{ti}")
                        cos4 = cos_sb[:, tt, :].rearrange("p (h j) -> p h j", h=4)
                        sin4 = sin_sb[:, tt, :].rearrange("p (h j) -> p h j", h=4)
                        for half in range(2):  # 0: q cols 0:256, 1: k cols
                            src = ps_qk[:, half * 256:(half + 1) * 256].rearrange(
                                "p (h j two) -> p two h j", j=32, two=2)
                            dst = qk_r[:, half * 256:(half + 1) * 256].rearrange(
                                "p (h pm j) -> p pm h j", pm=2, j=32)
                            ev, od = src[:, 0], src[:, 1]
                            t1 = rtpool.tile([128, 4, 32], f32, tag="t1")
                            t2 = rtpool.tile([128, 4, 32], f32, tag="t2")
                            nc.vector.tensor_mul(t1[:], od, sin4)
                            nc.vector.tensor_mul(dst[:, 0], ev, cos4)
                            nc.vector.tensor_sub(dst[:, 0], dst[:, 0], t1[:])
                            nc.vector.tensor_mul(t2[:], ev, sin4)
                            nc.vector.tensor_mul(dst[:, 1], od, cos4)
                            nc.vector.tensor_add(dst[:, 1], dst[:, 1], t2[:])
                        qkrs.append(qk_r)

                    # ---- transpose roped qk into qT/kT ----
                    for cc in range(4):
                        ps_t = psp.tile([128, 512], f32, tag="pp")
                        for ti in range(4):
                            nc.tensor.transpose(
                                ps_t[:, ti * 128:(ti + 1) * 128],
                                qkrs[ti][:, cc * 128:(cc + 1) * 128], ident[:])
                        dstbuf = qT if cc < 2 else kT
                        nc.vector.tensor_copy(
                            dstbuf[:, cc % 2, g * 512:(g + 1) * 512], ps_t[:])

                # ================= stage B: attention =================
                for qc in range(QC):
                    for hp in range(2):
                        O_A = pso.tile([128, 512], f32, tag="O", name="O_A")
                        O_B = pso.tile([128, 512], f32, tag="O", name="O_B")
                        lring = psl.tile([128, 3, 512], f32, tag="L")
                        ering = etpool.tile([128, 6, 512], f32r, tag="et")
                        for kt in range(KT):
                            sA, sB = (2 * kt) % 3, (2 * kt + 1) % 3
                            eA, eB = (2 * kt) % 6, (2 * kt + 1) % 6
                            nc.tensor.matmul(
                                lring[:, sA, :],
                                kT[0:64, hp, kt * 128:(kt + 1) * 128],
                                qT[0:64, hp, qc * 512:(qc + 1) * 512],
                                start=True, stop=True, tile_position=(0, 0))
                            nc.tensor.matmul(
                                lring[:, sB, :],
                                kT[64:128, hp, kt * 128:(kt + 1) * 128],
                                qT[64:128, hp, qc * 512:(qc + 1) * 512],
                                start=True, stop=True, tile_position=(64, 0))
                            nc.scalar.activation(ering[:, eA, :], lring[:, sA, :],
                                                 Exp, scale=0.125)
                            nc.scalar.activation(ering[:, eB, :], lring[:, sB, :],
                                                 Exp, scale=0.125)
                            nc.tensor.matmul(
                                O_A[0:65, :], v_sb[:, kt, 2 * hp, :], ering[:, eA, :],
                                start=(kt == 0), stop=(kt == KT - 1))
                            nc.tensor.matmul(
                                O_B[0:65, :], v_sb[:, kt, 2 * hp + 1, :], ering[:, eB, :],
                                start=(kt == 0), stop=(kt == KT - 1))
                        for (O_ps, odd) in ((O_A, 0), (O_B, 1)):
                            recip = spool.tile([1, 512], f32r, tag="rc")
                            with nc.allow_low_precision(
                                    reason="f32r reciprocal feeds f32r matmul"):
                                nc.vector.reciprocal(recip[:], O_ps[64:65, :])
                            nc.tensor.matmul(lring[0:64, odd, :], ones_sb[0:1, 0:64],
                                             recip[:], start=True, stop=True)
                            bc_sb = spool.tile([64, 512], f32, tag="bc")
                            nc.vector.tensor_copy(bc_sb[:], lring[0:64, odd, :])
                            if not odd:
                                nc.vector.tensor_mul(
                                    attn[0:64, hp, qc * 512:(qc + 1) * 512],
                                    O_ps[0:64, :], bc_sb[:])
                            else:
                                t_at = spool.tile([64, 512], f32r, tag="ta")
                                nc.vector.tensor_mul(t_at[:], O_ps[0:64, :], bc_sb[:])
                                nc.sync.dma_start(
                                    attn[64:128, hp, qc * 512:(qc + 1) * 512],
                                    t_at[:])

                    # ---- stage C: out-proj for this query chunk ----
                    for ti in range(4):
                        tt = qc * 4 + ti
                        y_t = ypool.tile([128, D], f32)
                        for ec in range(2):
                            ps_y = psp.tile([128, 512], f32, tag="pp")
                            nc.tensor.matmul(
                                ps_y[:], attn[:, 0, tt * 128:(tt + 1) * 128],
                                wout_sb[:, 0, ec * 512:(ec + 1) * 512],
                                start=True, stop=False)
                            nc.tensor.matmul(
                                ps_y[:], attn[:, 1, tt * 128:(tt + 1) * 128],
                                wout_sb[:, 1, ec * 512:(ec + 1) * 512],
                                start=False, stop=False)
                            nc.tensor.matmul(
                                ps_y[:], ones_sb[0:1, 0:128],
                                bout_sb[0:1, ec * 512:(ec + 1) * 512],
                                start=False, stop=True)
                            nc.vector.tensor_copy(
                                y_t[:, ec * 512:(ec + 1) * 512], ps_y[:])
                        nc.sync.dma_start(y_d[tt * 128:(tt + 1) * 128, :], y_t[:])

            if repeats == 1:
                body()
            else:
                with tc.For_i(0, repeats, 1) as _i:
                    body(_i)

    nc.compile()
    return nc


def _prep_in_maps(x, rope_cos, rope_sin, W_qkv, b_qkv, W_out, b_out):
    f32 = np.float32
    W3 = np.asarray(W_qkv, dtype=f32).reshape(D, 16, 3, HD)  # [f, head, qkv, d]
    b3 = np.asarray(b_qkv, dtype=f32).reshape(16, 3, HD)
    cos_r = np.ascontiguousarray(np.tile(np.asarray(rope_cos, dtype=f32), (1, 4)))
    sin_r = np.ascontiguousarray(np.tile(np.asarray(rope_sin, dtype=f32), (1, 4)))
    ones = np.ones((1, 128), dtype=f32)
    onescol = np.ones((128, 64), dtype=f32)
    W_out = np.asarray(W_out, dtype=f32)
    b_out = np.asarray(b_out, dtype=f32)
    x = np.asarray(x, dtype=f32)

    in_maps = []
    for c in range(N_CORES):
        b, hg = divmod(c, 4)
        hs = slice(hg * H_LOC, (hg + 1) * H_LOC)
        wq = W3[:, hs, 0, :].reshape(D, 256)
        wk = W3[:, hs, 1, :].reshape(D, 256)
        wv = W3[:, hs, 2, :].reshape(D, 256)
        bq = b3[hs, 0, :].reshape(1, 256)
        bk = b3[hs, 1, :].reshape(1, 256)
        bv = b3[hs, 2, :].reshape(1, 256)
        in_maps.append({
            "x": np.ascontiguousarray(x[b]),
            "cosr": cos_r, "sinr": sin_r,
            "wqk": np.ascontiguousarray(np.concatenate([wq, wk], axis=1)),
            "wv": np.ascontiguousarray(wv),
            "wout": np.ascontiguousarray(W_out[hg * 256:(hg + 1) * 256, :]),
            "bqk": np.ascontiguousarray(np.concatenate([bq, bk], axis=1)),
            "bv": np.ascontiguousarray(bv),
            "bout": (np.ascontiguousarray(b_out.reshape(1, D)) if hg == 0
                     else np.zeros((1, D), dtype=f32)),
            "ones": ones, "onescol": onescol,
        })
    return in_maps


def kernel(x, rope_cos, rope_sin, W_qkv, b_qkv, W_out, b_out):
    from concourse.bass_utils import run_bass_kernel_spmd

    if "nc" not in _CACHED:
        _CACHED["nc"] = build_nc(1)
    nc = _CACHED["nc"]
    in_maps = _prep_in_maps(x, rope_cos, rope_sin, W_qkv, b_qkv, W_out, b_out)
    res = run_bass_kernel_spmd(nc, in_maps, list(range(N_CORES)))
    B = x.shape[0]
    out = np.zeros((B, S, D), dtype=np.float32)
    for c in range(N_CORES):
        b = c // 4
        out[b] += res.results[c]["y"]
    return out


# revision 11
# speedup vs baseline: 15.7770x; 1.0240x over previous
"""Trainium2 Bass kernel for nn_Attention (B=2, S=2048, D=1024, H=16).

Sharding: 8 cores = 2 batches x 4 head-groups (4 heads per core).
Each core computes QKV projection for its batch restricted to its 4 heads,
full (non-causal) attention for those heads, and a partial output
projection over its 256 channels. The host sums the 4 partial outputs per
batch (the out-proj bias is fed only to head-group 0's cores).

Device dataflow (per core), matmuls in float32r (~1.5e-4 rel err, 4x the
fp32 PE throughput):
  1. x [2048,1024] -> PE-transpose -> xT [f, tok] (f32r)
  2. qk proj (natural [tok, 512]): 8 accum matmuls + K=1 ones-row bias
     -> RoPE on DVE (writes evens/odds-permuted d order) -> PE-transpose
     -> qT/kT [128, 2 head-pair chunks, 2048]
     v proj -> V [kt, 16 tiles, 4 heads, 65] with a ones column (col 64)
  3. attention per (qt-chunk, head-pair): logitsT = kT.T @ qT (row-packed
     2 heads at K=64), exp on ACT (1/8 scale fused), OT accum =
     V_aug.T @ ET over 16 kt tiles; row 64 of OT = softmax sums.
  4. divide: recip(sums) -> PE outer-product broadcast -> DVE multiply;
     odd head rows shifted to partitions 64:128 via SBUF-SBUF DMA.
  5. out-proj: y[tok,e] accum over 2 channel chunks + K=1 ones-row bias.
"""

import numpy as np

S = 2048
D = 1024
HD = 64
H_LOC = 4  # heads per core
N_CORES = 8
TT = 16  # token tiles of 128
G = 4    # token groups of 512
QC = 4   # query chunks of 512
KT = 16  # key tiles of 128

_CACHED = {}


def build_nc(repeats: int = 1):
    import concourse.bass as bass
    import concourse.mybir as mybir
    from concourse import bacc
    from concourse.tile import TileContext
    from concourse.masks import make_identity

    f32 = mybir.dt.float32
    f32r = mybir.dt.float32r
    Exp = mybir.ActivationFunctionType.Exp

    nc = bacc.Bacc("TRN2", target_bir_lowering=False, debug=False,
                   num_devices=N_CORES)

    x_d = nc.dram_tensor("x", [S, D], f32, kind="ExternalInput")
    cos_d = nc.dram_tensor("cosr", [S, 32], f32, kind="ExternalInput")
    sin_d = nc.dram_tensor("sinr", [S, 32], f32, kind="ExternalInput")
    wqk_d = nc.dram_tensor("wqk", [D, 512], f32r, kind="ExternalInput")
    wv_d = nc.dram_tensor("wv", [D, 256], f32r, kind="ExternalInput")
    wout_d = nc.dram_tensor("wout", [256, D], f32r, kind="ExternalInput")
    bqk_d = nc.dram_tensor("bqk", [1, 512], f32r, kind="ExternalInput")
    bv_d = nc.dram_tensor("bv", [1, 256], f32r, kind="ExternalInput")
    bout_d = nc.dram_tensor("bout", [1, D], f32r, kind="ExternalInput")
    ones_d = nc.dram_tensor("ones", [1, 128], f32r, kind="ExternalInput")
    onescol_d = nc.dram_tensor("onescol", [128, 64], f32r,
                               kind="ExternalInput")
    y_d = nc.dram_tensor("y", [S, D], f32, kind="ExternalOutput")

    with TileContext(nc) as tc:
        with (
            tc.tile_pool(name="const", bufs=1) as cpool,
            tc.tile_pool(name="xin", bufs=1) as xpool,
            tc.tile_pool(name="xt", bufs=1) as xtpool,
            tc.tile_pool(name="qkr", bufs=1) as qkrpool,
            tc.tile_pool(name="rtmp", bufs=2) as rtpool,
            tc.tile_pool(name="big", bufs=1) as bigpool,
            tc.tile_pool(name="et", bufs=2) as etpool,
            tc.tile_pool(name="yt", bufs=2) as ypool,
            tc.tile_pool(name="sml", bufs=2) as spool,
            tc.tile_pool(name="psp", bufs=1, space="PSUM") as psp,
            tc.tile_pool(name="psl", bufs=1, space="PSUM") as psl,
            tc.tile_pool(name="pso", bufs=2, space="PSUM") as pso,
        ):
            # ---- constants / weights ----
            wqk_sb = cpool.tile([128, 8, 512], f32r)
            wv_sb = cpool.tile([128, 8, 256], f32r)
            wout_sb = cpool.tile([128, 2, D], f32r)
            cos_sb = cpool.tile([128, TT, 32], f32)
            sin_sb = cpool.tile([128, TT, 32], f32)
            bqk_sb = cpool.tile([1, 512], f32r)
            bv_sb = cpool.tile([1, 256], f32r)
            bout_sb = cpool.tile([1, D], f32r)
            ones_sb = cpool.tile([1, 128], f32r)
            onescol_sb = cpool.tile([128, 64], f32r)
            ident = cpool.tile([128, 128], f32)

            nc.sync.dma_start(wqk_sb[:], wqk_d.ap().rearrange("(i p) c -> p i c", p=128))
            nc.sync.dma_start(wv_sb[:], wv_d.ap().rearrange("(i p) c -> p i c", p=128))
            nc.sync.dma_start(wout_sb[:], wout_d.ap().rearrange("(i p) c -> p i c", p=128))
            nc.sync.dma_start(cos_sb[:], cos_d.ap().rearrange("(t p) c -> p t c", p=128))
            nc.sync.dma_start(sin_sb[:], sin_d.ap().rearrange("(t p) c -> p t c", p=128))
            nc.sync.dma_start(bqk_sb[:], bqk_d[:])
            nc.sync.dma_start(bv_sb[:], bv_d[:])
            nc.sync.dma_start(bout_sb[:], bout_d[:])
            nc.sync.dma_start(ones_sb[:], ones_d[:])
            nc.sync.dma_start(onescol_sb[:], onescol_d[:])
            make_identity(nc, ident[:])

            def body(_iv=None):
                qT = bigpool.tile([128, 2, S], f32r, tag="qT")
                kT = bigpool.tile([128, 2, S], f32r, tag="kT")
                attn = bigpool.tile([128, 2, S], f32r, tag="attn")
                v_sb = bigpool.tile([128, TT, H_LOC, 65], f32r, tag="v")
                nc.vector.tensor_copy(
                    v_sb[:, :, :, 64:65],
                    onescol_sb[:].rearrange("p (t h o) -> p t h o", h=H_LOC, o=1))

                # ================= stage A: projections =================
                import concourse.bass as bass_mod

                def bcast8(ap):
                    return bass_mod.AP(ap.tensor, ap.offset,
                                       [ap.ap[0], [0, 8], ap.ap[1]])

                for g in range(G):
                    ring = psp.tile([128, 3, 512], f32, tag="ring", name=f"ringA{g}")
                    rs = [0]

                    def rslice(n=512):
                        s = rs[0] % 3
                        rs[0] += 1
                        return ring[:, s, 0:n]

                    xts = []
                    for ti in range(4):
                        tt = g * 4 + ti
                        x_t = xpool.tile([128, D], f32, tag=f"x{ti}")
                        nc.sync.dma_start(x_t[:], x_d[tt * 128:(tt + 1) * 128, :])
                        xts.append(x_t)

                    xT_g = xtpool.tile([128, 8, 512], f32r)
                    for fc in range(8):
                        ps_x = rslice()
                        for ti in range(4):
                            nc.tensor.transpose(
                                ps_x[:, ti * 128:(ti + 1) * 128],
                                xts[ti][:, fc * 128:(fc + 1) * 128], ident[:])
                        nc.vector.tensor_copy(xT_g[:, fc, :], ps_x)

                    qkrs = []
                    for ti in range(4):
                        tt = g * 4 + ti
                        # ---- v projection ----
                        ps_v = rslice(256)
                        for fc in range(8):
                            nc.tensor.matmul(
                                ps_v, xT_g[:, fc, ti * 128:(ti + 1) * 128],
                                wv_sb[:, fc, :],
                                start=(fc == 0), stop=False)
                        nc.tensor.matmul(ps_v, ones_sb[0:1, 0:128], bv_sb[:],
                                         start=False, stop=True)
                        nc.vector.tensor_copy(
                            v_sb[:, tt, :, 0:64],
                            ps_v.rearrange("p (h d) -> p h d", h=H_LOC))

                        # ---- qk projection (natural layout) ----
                        ps_qk = rslice()
                        for fc in range(8):
                            nc.tensor.matmul(
                                ps_qk, xT_g[:, fc, ti * 128:(ti + 1) * 128],
                                wqk_sb[:, fc, :],
                                start=(fc == 0), stop=False)
                        nc.tensor.matmul(ps_qk, ones_sb[0:1, 0:128], bqk_sb[:],
                                         start=False, stop=True)

                        # ---- rope (6 batched DVE ops, step-0 cos bcast) ----
                        qk_r = qkrpool.tile([128, 512], f32, tag=f"qkr{ti}")
                        cos8 = bcast8(cos_sb[:, tt, :])
                        sin8 = bcast8(sin_sb[:, tt, :])
                        srcr = ps_qk.rearrange("p (g j two) -> p two g j",
                                               g=8, j=32, two=2)
                        dstr = qk_r[:].rearrange("p (g pm j) -> p pm g j",
                                                 pm=2, j=32)
                        ev, od = srcr[:, 0], srcr[:, 1]
                        t1 = rtpool.tile([128, 8, 32], f32, tag="t1")
                        t2 = rtpool.tile([128, 8, 32], f32, tag="t2")
                        nc.vector.tensor_mul(t1[:], od, sin8)
                        nc.vector.tensor_mul(dstr[:, 0], ev, cos8)
                        nc.vector.tensor_sub(dstr[:, 0], dstr[:, 0], t1[:])
                        nc.vector.tensor_mul(t2[:], ev, sin8)
                        nc.vector.tensor_mul(dstr[:, 1], od, cos8)
                        nc.vector.tensor_add(dstr[:, 1], dstr[:, 1], t2[:])
                        qkrs.append(qk_r)

                    # ---- transpose roped qk into qT/kT ----
                    for cc in range(4):
                        ps_t = rslice()
                        for ti in range(4):
                            nc.tensor.transpose(
                                ps_t[:, ti * 128:(ti + 1) * 128],
                                qkrs[ti][:, cc * 128:(cc + 1) * 128], ident[:])
                        dstbuf = qT if cc < 2 else kT
                        nc.vector.tensor_copy(
                            dstbuf[:, cc % 2, g * 512:(g + 1) * 512], ps_t)

                # ================= stage B: attention =================
                for qc in range(QC):
                    for hp in range(2):
                        O_A = pso.tile([128, 512], f32, tag="O", name="O_A")
                        O_B = pso.tile([128, 512], f32, tag="O", name="O_B")
                        lring = psl.tile([128, 3, 512], f32, tag="L")
                        ering = etpool.tile([128, 6, 512], f32r, tag="et")
                        for kt in range(KT):
                            sA, sB = (2 * kt) % 3, (2 * kt + 1) % 3
                            eA, eB = (2 * kt) % 6, (2 * kt + 1) % 6
                            nc.tensor.matmul(
                                lring[:, sA, :],
                                kT[0:64, hp, kt * 128:(kt + 1) * 128],
                                qT[0:64, hp, qc * 512:(qc + 1) * 512],
                                start=True, stop=True, tile_position=(0, 0))
                            nc.tensor.matmul(
                                lring[:, sB, :],
                                kT[64:128, hp, kt * 128:(kt + 1) * 128],
                                qT[64:128, hp, qc * 512:(qc + 1) * 512],
                                start=True, stop=True, tile_position=(64, 0))
                            nc.scalar.activation(ering[:, eA, :], lring[:, sA, :],
                                                 Exp, scale=0.125)
                            nc.scalar.activation(ering[:, eB, :], lring[:, sB, :],
                                                 Exp, scale=0.125)
                            nc.tensor.matmul(
                                O_A[0:65, :], v_sb[:, kt, 2 * hp, :], ering[:, eA, :],
                                start=(kt == 0), stop=(kt == KT - 1))
                            nc.tensor.matmul(
                                O_B[0:65, :], v_sb[:, kt, 2 * hp + 1, :], ering[:, eB, :],
                                start=(kt == 0), stop=(kt == KT - 1))
                        for (O_ps, odd) in ((O_A, 0), (O_B, 1)):
                            recip = spool.tile([1, 512], f32r, tag="rc")
                            with nc.allow_low_precision(
                                    reason="f32r reciprocal feeds f32r matmul"):
                                nc.vector.reciprocal(recip[:], O_ps[64:65, :])
                            nc.tensor.matmul(lring[0:64, odd, :], ones_sb[0:1, 0:64],
                                             recip[:], start=True, stop=True)
                            bc_sb = spool.tile([64, 512], f32, tag="bc")
                            nc.vector.tensor_copy(bc_sb[:], lring[0:64, odd, :])
                            if not odd:
                                nc.vector.tensor_mul(
                                    attn[0:64, hp, qc * 512:(qc + 1) * 512],
                                    O_ps[0:64, :], bc_sb[:])
                            else:
                                t_at = spool.tile([64, 512], f32r, tag="ta")
                                nc.vector.tensor_mul(t_at[:], O_ps[0:64, :], bc_sb[:])
                                nc.sync.dma_start(
                                    attn[64:128, hp, qc * 512:(qc + 1) * 512],
                                    t_at[:])

                    # ---- stage C: out-proj for this query chunk ----
                    ringc = psp.tile([128, 3, 512], f32, tag="ring",
                                     name=f"ringC{qc}")
                    for ti in range(4):
                        tt = qc * 4 + ti
                        y_t = ypool.tile([128, D], f32)
                        for ec in range(2):
                            ps_y = ringc[:, (ti * 2 + ec) % 3, :]
                            nc.tensor.matmul(
                                ps_y, attn[:, 0, tt * 128:(tt + 1) * 128],
                                wout_sb[:, 0, ec * 512:(ec + 1) * 512],
                                start=True, stop=False)
                            nc.tensor.matmul(
                                ps_y, attn[:, 1, tt * 128:(tt + 1) * 128],
                                wout_sb[:, 1, ec * 512:(ec + 1) * 512],
                                start=False, stop=False)
                            nc.tensor.matmul(
                                ps_y, ones_sb[0:1, 0:128],
                                bout_sb[0:1, ec * 512:(ec + 1) * 512],
                                start=False, stop=True)
                            nc.vector.tensor_copy(
                                y_t[:, ec * 512:(ec + 1) * 512], ps_y)
                        nc.sync.dma_start(y_d[tt * 128:(tt + 1) * 128, :], y_t[:])

            if repeats == 1:
                body()
            else:
                with tc.For_i(0, repeats, 1) as _i:
                    body(_i)

    nc.compile()
    return nc


def _prep_in_maps(x, rope_cos, rope_sin, W_qkv, b_qkv, W_out, b_out):
    f32 = np.float32
    W3 = np.asarray(W_qkv, dtype=f32).reshape(D, 16, 3, HD)  # [f, head, qkv, d]
    b3 = np.asarray(b_qkv, dtype=f32).reshape(16, 3, HD)
    cos_r = np.ascontiguousarray(np.asarray(rope_cos, dtype=f32))
    sin_r = np.ascontiguousarray(np.asarray(rope_sin, dtype=f32))
    ones = np.ones((1, 128), dtype=f32)
    onescol = np.ones((128, 64), dtype=f32)
    W_out = np.asarray(W_out, dtype=f32)
    b_out = np.asarray(b_out, dtype=f32)
    x = np.asarray(x, dtype=f32)

    in_maps = []
    for c in range(N_CORES):
        b, hg = divmod(c, 4)
        hs = slice(hg * H_LOC, (hg + 1) * H_LOC)
        wq = W3[:, hs, 0, :].reshape(D, 256)
        wk = W3[:, hs, 1, :].reshape(D, 256)
        wv = W3[:, hs, 2, :].reshape(D, 256)
        bq = b3[hs, 0, :].reshape(1, 256)
        bk = b3[hs, 1, :].reshape(1, 256)
        bv = b3[hs, 2, :].reshape(1, 256)
        in_maps.append({
            "x": np.ascontiguousarray(x[b]),
            "cosr": cos_r, "sinr": sin_r,
            "wqk": np.ascontiguousarray(np.concatenate([wq, wk], axis=1)),
            "wv": np.ascontiguousarray(wv),
            "wout": np.ascontiguousarray(W_out[hg * 256:(hg + 1) * 256, :]),
            "bqk": np.ascontiguousarray(np.concatenate([bq, bk], axis=1)),
            "bv": np.ascontiguousarray(bv),
            "bout": (np.ascontiguousarray(b_out.reshape(1, D)) if hg == 0
                     else np.zeros((1, D), dtype=f32)),
            "ones": ones, "onescol": onescol,
        })
    return in_maps


def kernel(x, rope_cos, rope_sin, W_qkv, b_qkv, W_out, b_out):
    from concourse.bass_utils import run_bass_kernel_spmd

    if "nc" not in _CACHED:
        _CACHED["nc"] = build_nc(1)
    nc = _CACHED["nc"]
    in_maps = _prep_in_maps(x, rope_cos, rope_sin, W_qkv, b_qkv, W_out, b_out)
    res = run_bass_kernel_spmd(nc, in_maps, list(range(N_CORES)))
    B = x.shape[0]
    out = np.zeros((B, S, D), dtype=np.float32)
    for c in range(N_CORES):
        b = c // 4
        out[b] += res.results[c]["y"]
    return out
